# revision 32
# baseline (speedup 1.0000x reference)
"""Trainium2 Bass kernel for nn_BCE_topK_loss_landmark.

Computes mean(top_k(BCE_with_logits(net_output, scattered_target), k=10%))
over each (b, c) row of a [B=2, C=8, D=64, H=192, W=192] volume.

Algorithm (per (b,c) row of N = D*H*W = 2,359,296 elements, n = 235,930):
  - target is zero outside a tiny 15^3 patch, so loss = softplus(x) except
    inside the patch where loss = softplus(x) - x*tgt.
  - mean of top-n values = (sum relu(loss - t) + n*t) / n for any threshold
    t in [v_{n+1}, v_n]; the estimator's error is second order in (t - v_n),
    so a sampled-quantile t (accuracy ~1e-2) gives ~1e-4 relative error.
    sum relu(loss - t) = sum max(loss, t) - N*t, which maps onto a single
    tensor_scalar(op0=max, accum op1=add) per tile.
  - Phase S: sample 9216 elements of the row, count sample > a_j for a
    fixed 128-point threshold grid (immediates), pick t = largest grid
    point whose count >= n * 9216/N.  All counts/selection on device.
  - Phase M: stream the full row once: softplus via ACT (Exp then
    Ln(e+1)), then one DVE tensor_scalar (subtract t, max 0) with
    accum_out per-partition partial sums.
  - Phase P: exact patch correction on the 3375 patch elements
    (host pre-gathers patch x/tgt; bboxes known on host).
  - Host sums the 16 per-row partials from the 8 cores and divides.

Sharding: data-parallel over B*C = 16 rows, 2 rows per core, 8 cores.
"""

import os
import numpy as np

B, C, D, H, W, P = 2, 8, 64, 192, 192, 15
NROW = D * H * W          # 2359296
RTOT = B * C              # 16
NCORES = 8
RPC = RTOT // NCORES      # 2 rows per core
NTOP = max(1, round(NROW * 10 / 100))  # 235930

PART = 128
FROW = NROW // PART       # 18432
FT = 4608                 # free-dim tile size
NTILE = FROW // FT        # 4 tiles per row

# Sampling phase: 128 partitions x 4 chunks x 16 contiguous = 9216 samples
SP_CH = 4
SP_EL = 16
SPP = SP_CH * SP_EL       # 72 samples per partition
NS = PART * SPP           # 9216
NS_TARGET = NTOP * NS / NROW  # 921.60 (fractional is fine for compares)
PVOL = P * P * P          # 3375
NGRID = 64                # threshold grid points per row


def _softplus64(v):
    return np.log1p(np.exp(-np.abs(v))) + np.maximum(v, 0.0)


def _make_grid():
    """128 x-space thresholds: dense around the expected 90th percentile of
    N(0,1) (1.2816), coarse tails so any distribution shift still brackets."""
    lo = np.array([-4.0, -1.0, 0.0, 0.5, 0.8, 1.0])
    fine = 1.03 + 0.01 * np.arange(48)        # 1.03 .. 1.50
    hi = np.array([1.52, 1.56, 1.62, 1.7, 1.85, 2.1, 2.6, 3.3, 4.2, 5.2])
    gx = np.concatenate([lo, fine, hi])
    assert gx.size == NGRID
    gl = _softplus64(gx).astype(np.float32)   # loss-space value per grid pt
    return gx.astype(np.float32), gl


def _build_program():
    import concourse.bass as bass  # noqa: F401
    import concourse.mybir as mybir
    from concourse import tile
    from concourse.bacc import Bacc

    f32 = mybir.dt.float32
    AF = mybir.ActivationFunctionType
    OP = mybir.AluOpType
    X = mybir.AxisListType.X

    gx, _gl = _make_grid()

    # Bacc (not plain Bass): its compile pipeline splits multi-sem waits
    # into EventSemaphore chains (TRN2 allows 1 wait/instruction) and
    # auto-inserts gpsimd library + ACT table loads.
    nc = Bacc()
    xrows = nc.declare_dram_parameter("xrows", [RPC, NROW], f32, isOutput=False)
    # patches[r, d, 0, :] = x patch slice, patches[r, d, 1, :] = target patch
    patches = nc.declare_dram_parameter("patches", [RPC, P, 2, P * P], f32,
                                        isOutput=False)
    gridl = nc.declare_dram_parameter("gridl", [RPC * NGRID], f32,
                                      isOutput=False)
    partials = nc.declare_dram_parameter("partials", [RPC], f32, isOutput=True)

    with tile.TileContext(nc) as tc:
        with tc.tile_pool(name="small", bufs=1) as small, \
             tc.tile_pool(name="psum", bufs=1, space="PSUM") as psum, \
             tc.tile_pool(name="xp", bufs=RPC * NTILE) as xpool:

            ones128 = small.tile([PART, 1], f32)
            nc.vector.memset(ones128[:], 1.0)
            ones1 = small.tile([1, PART], f32)
            nc.vector.memset(ones1[:], 1.0)
            ones15 = small.tile([P, 1], f32)
            nc.vector.memset(ones15[:], 1.0)

            # ---------- Phase S: sample rows, find per-row threshold ----------
            samp = small.tile([PART, RPC * SPP], f32)
            for r in range(RPC):
                src = xrows[r].rearrange("(p c i) -> p c i", p=PART, c=SP_CH)
                dst = samp[:, r * SPP:(r + 1) * SPP].rearrange(
                    "p (c i) -> p c i", c=SP_CH)
                nc.sync.dma_start(out=dst, in_=src[:, :, 0:SP_EL])

            # per-row count/scratch tiles: keeps the first count op of each
            # row at a single wait (its own sample DMA)
            ctot = small.tile([1, RPC * NGRID], f32)
            for r in range(RPC):
                counts = small.tile([PART, NGRID], f32, tag=f"counts{r}")
                cscr = small.tile([PART, SPP], f32, tag=f"cscr{r}")
                s_ap = samp[:, r * SPP:(r + 1) * SPP]
                for j in range(NGRID):
                    nc.vector.tensor_scalar(
                        out=cscr[:], in0=s_ap, scalar1=float(gx[j]),
                        scalar2=None, op0=OP.is_gt, op1=OP.add,
                        accum_out=counts[:, j:j + 1])
                # cross-partition count totals via ones-matmul -> [1, 128]
                ctot_ps = psum.tile([1, NGRID], f32, tag=f"ctot{r}")
                nc.tensor.matmul(ctot_ps[:], ones128[:], counts[:],
                                 start=True, stop=True)
                nc.vector.tensor_copy(out=ctot[0:1, r * NGRID:(r + 1) * NGRID],
                                      in_=ctot_ps[:])

            maskv = small.tile([1, RPC * NGRID], f32)
            nc.vector.tensor_scalar(
                out=maskv[:], in0=ctot[:], scalar1=float(NS_TARGET),
                scalar2=None, op0=OP.is_ge)

            gl0 = small.tile([1, RPC * NGRID], f32)
            nc.sync.dma_start(out=gl0[:], in_=gridl[:])
            # stage the grid through a DVE copy so `tv` only has
            # same-engine deps (1-wait-per-instruction HW limit)
            gl0s = small.tile([1, RPC * NGRID], f32)
            nc.vector.tensor_copy(out=gl0s[:], in_=gl0[:])
            tv = small.tile([1, RPC * NGRID], f32)
            nc.vector.tensor_tensor(out=tv[:], in0=maskv[:], in1=gl0s[:],
                                    op=OP.mult)

            trow = small.tile([1, RPC], f32)  # per-row threshold (loss space)
            for r in range(RPC):
                nc.vector.tensor_reduce(
                    out=trow[:, r:r + 1], in_=tv[0:1, r * NGRID:(r + 1) * NGRID],
                    axis=X, op=OP.max)

            # broadcast per-row threshold to all 128 partitions (K=1 matmul)
            tb_ps = psum.tile([PART, RPC], f32)
            nc.tensor.matmul(tb_ps[:], ones1[:], trow[:],
                             start=True, stop=True)
            # copy PSUM->SBUF on ACT so downstream DVE consumers' tbc dep
            # shares the Activation semaphore with their data dep (1 wait)
            tbc = small.tile([PART, RPC], f32)
            nc.scalar.activation(out=tbc[:], in_=tb_ps[:], func=AF.Copy)

            # ---------- Phase M: stream full rows ----------
            acc = small.tile([PART, RPC * NTILE], f32)
            for r in range(RPC):
                xrv = xrows[r].rearrange("(p f) -> p f", p=PART)
                for k in range(NTILE):
                    # whole chain is in-place on xt: keeps every ACT/DVE
                    # instruction at <=1 cross-engine wait (HW wait-slot cap)
                    xt = xpool.tile([PART, FT], f32, tag="xt")
                    nc.sync.dma_start(out=xt[:], in_=xrv[:, k * FT:(k + 1) * FT])
                    nc.scalar.activation(out=xt[:], in_=xt[:], func=AF.Exp)
                    nc.scalar.activation(out=xt[:], in_=xt[:], func=AF.Ln,
                                         bias=1.0)
                    # accum col = sum_f max(softplus(x), t); N*t removed later
                    nc.vector.tensor_scalar(
                        out=xt[:], in0=xt[:], scalar1=tbc[:, r:r + 1],
                        scalar2=None, op0=OP.max, op1=OP.add,
                        accum_out=acc[:, r * NTILE + k:r * NTILE + k + 1])

            # ---------- Phase P: exact patch correction ----------
            pd2 = small.tile([P, RPC], f32)
            for r in range(RPC):
                # one DMA per row brings interleaved x/target patch data, so
                # every consumer has a single-queue DMA dependency
                pt = small.tile([P, 2 * P * P], f32, tag=f"pt{r}")
                nc.sync.dma_start(out=pt[:], in_=patches[r])
                xpt = pt[:, 0:P * P]
                tpt = pt[:, P * P:2 * P * P]
                ept = small.tile([P, P * P], f32, tag=f"ept{r}")
                spt = small.tile([P, P * P], f32, tag=f"spt{r}")
                nc.scalar.activation(out=ept[:], in_=xpt, func=AF.Exp)
                nc.scalar.activation(out=spt[:], in_=ept[:], func=AF.Ln,
                                     bias=1.0)
                mt = small.tile([P, P * P], f32, tag=f"mt{r}")
                nc.vector.tensor_tensor(out=mt[:], in0=xpt, in1=tpt,
                                        op=OP.mult)
                # stage spt through a DVE copy (single ACT wait) so the
                # subtract below carries only same-engine deps
                spts = small.tile([P, P * P], f32, tag=f"spts{r}")
                nc.vector.tensor_copy(out=spts[:], in_=spt[:])
                lpt = small.tile([P, P * P], f32, tag=f"lpt{r}")
                nc.vector.tensor_tensor(out=lpt[:], in0=spts[:], in1=mt[:],
                                        op=OP.subtract)
                # dS = sum max(lp,t) - sum max(sp,t)  (N*t terms cancel)
                pacc = small.tile([P, 2], f32, tag=f"pacc{r}")
                pscr = small.tile([P, P * P], f32, tag=f"pscr{r}")
                nc.vector.tensor_scalar(
                    out=pscr[:], in0=lpt[:], scalar1=tbc[0:P, r:r + 1],
                    scalar2=None, op0=OP.max, op1=OP.add,
                    accum_out=pacc[:, 0:1])
                nc.vector.tensor_scalar(
                    out=pscr[:], in0=spt[:], scalar1=tbc[0:P, r:r + 1],
                    scalar2=None, op0=OP.max, op1=OP.add,
                    accum_out=pacc[:, 1:2])
                nc.vector.tensor_tensor(out=pd2[:, r:r + 1], in0=pacc[:, 0:1],
                                        in1=pacc[:, 1:2], op=OP.subtract)
            pdel_ps = psum.tile([1, RPC], f32)
            nc.tensor.matmul(pdel_ps[:], ones15[:], pd2[:],
                             start=True, stop=True)
            pdelta = small.tile([1, RPC], f32)
            nc.vector.tensor_copy(out=pdelta[:], in_=pdel_ps[:])

            # ---------- Final assembly ----------
            macc = small.tile([PART, RPC], f32)
            for r in range(RPC):
                nc.vector.tensor_reduce(
                    out=macc[:, r:r + 1],
                    in_=acc[:, r * NTILE:(r + 1) * NTILE], axis=X, op=OP.add)
            # subtract FROW*t per partition BEFORE the cross-partition sum so
            # we sum small residuals (f32-friendly): sum relu = sum max - N*t
            tf = small.tile([PART, RPC], f32)
            nc.vector.tensor_scalar(out=tf[:], in0=tbc[:], scalar1=float(FROW),
                                    scalar2=None, op0=OP.mult)
            macc2 = small.tile([PART, RPC], f32)
            nc.vector.tensor_tensor(out=macc2[:], in0=macc[:], in1=tf[:],
                                    op=OP.subtract)
            mt_ps = psum.tile([1, RPC], f32)
            nc.tensor.matmul(mt_ps[:], ones128[:], macc2[:],
                             start=True, stop=True)
            mtot = small.tile([1, RPC], f32)
            nc.vector.tensor_copy(out=mtot[:], in_=mt_ps[:])
            nt = small.tile([1, RPC], f32)
            nc.vector.tensor_scalar(out=nt[:], in0=trow[:],
                                    scalar1=float(NTOP), scalar2=None,
                                    op0=OP.mult)
            s1 = small.tile([1, RPC], f32)
            nc.vector.tensor_tensor(out=s1[:], in0=mtot[:],
                                    in1=pdelta[:], op=OP.add)
            outsb = small.tile([1, RPC], f32)
            nc.vector.tensor_tensor(out=outsb[:], in0=s1[:], in1=nt[:],
                                    op=OP.add)
            nc.gpsimd.dma_start(out=partials[:], in_=outsb[0:1, :])
    nc.finalize()
    return nc


def _make_in_maps(net_output, target_structure, bboxes):
    _gx, gl = _make_grid()
    xf = net_output.reshape(RTOT, NROW)
    in_maps = []
    for core in range(NCORES):
        xr = np.ascontiguousarray(xf[core * RPC:(core + 1) * RPC])
        pts = np.zeros((RPC, P, 2, P * P), np.float32)
        for i in range(RPC):
            row = core * RPC + i
            b, c = divmod(row, C)
            d0, h0, w0 = (int(v) for v in bboxes[b, c])
            pts[i, :, 0, :] = net_output[b, c, d0:d0 + P, h0:h0 + P,
                                         w0:w0 + P].reshape(P, P * P)
            pts[i, :, 1, :] = target_structure[b].reshape(P, P * P)
        in_maps.append({"xrows": xr, "patches": pts,
                        "gridl": np.tile(gl, RPC)})
    return in_maps


def kernel(net_output, target_structure, bboxes):
    net_output = np.ascontiguousarray(np.asarray(net_output), np.float32)
    target_structure = np.ascontiguousarray(np.asarray(target_structure),
                                            np.float32)
    bboxes = np.asarray(bboxes)

    from concourse.bass_utils import run_bass_kernel_spmd

    nc = _build_program()
    in_maps = _make_in_maps(net_output, target_structure, bboxes)
    trace = bool(os.environ.get("KERNEL_TRACE"))
    res = run_bass_kernel_spmd(nc, in_maps, list(range(NCORES)), trace=trace)
    if trace:
        print("HW exec time:", res.exec_time_ns, "ns")
    total = 0.0
    for i in range(NCORES):
        total += float(np.asarray(res.results[i]["partials"],
                                  dtype=np.float64).sum())
    return np.float32(total / (RTOT * NTOP))
